# revision 14
# baseline (speedup 1.0000x reference)
"""Trainium2 Bass kernel for nn_Attention: 16-head attention layer, B=2, S=2048, H=1024.

Strategy (Megatron-style tensor parallel over heads, 8 cores x 2 heads):
  - Host transposes hidden_states once (XT [H, B*S]) and pre-rounds all matmul
    inputs to fp32r (TF32-like: 11-bit mantissa) so every matmul runs at the
    full 1-cycle/row PE rate with fp32 accumulation.
  - Each core computes its 2 heads' q/k/v via XT @ its W slice (transposed
    layout), attention with softmax folded as exp -> matmul-rowsum -> late
    normalization, then a partial dense projection over its 128 ctx columns.
  - Host sums the 8 partial dense outputs and adds dense_b.

All computed on device except the final 8-way partial reduction (done at
gather time on host, per the Megatron all-reduce-after-dense recipe).
"""
import os
import numpy as np

B, S, H, NH = 2, 2048, 1024, 16
HD = H // NH            # 64
BS = B * S              # 4096
NCORES = 8
ROWS_PER_CORE = 3 * HD * 2   # 384 qkv rows per core
DPC = 2 * HD                 # 128 ctx/dense columns per core

_CACHE = {}


def _round_fp32r(x):
    bits = np.ascontiguousarray(x, dtype=np.float32).view(np.uint32)
    lsb = (bits >> np.uint32(12)) & np.uint32(1)
    return ((bits + np.uint32(0x7FF) + lsb) & np.uint32(0xFFFFF000)).view(np.float32)


def _build_program():
    import concourse.mybir as mybir
    import concourse.tile as tile
    from concourse import bacc

    F32 = mybir.dt.float32
    F32R = mybir.dt.float32r
    Act = mybir.ActivationFunctionType

    nc = bacc.Bacc("TRN2", target_bir_lowering=False, debug=False,
                   num_devices=NCORES)
    xt = nc.dram_tensor("xt", [H, BS], F32R, kind="ExternalInput").ap()
    w1t = nc.dram_tensor("w1t", [H, ROWS_PER_CORE], F32R, kind="ExternalInput").ap()
    b1 = nc.dram_tensor("b1", [128, 3], F32, kind="ExternalInput").ap()
    w2t0 = nc.dram_tensor("w2t0", [HD, H], F32R, kind="ExternalInput").ap()
    w2t1 = nc.dram_tensor("w2t1", [HD, H], F32R, kind="ExternalInput").ap()
    eye2 = nc.dram_tensor("eye2", [128, HD], F32R, kind="ExternalInput").ap()
    ones2 = nc.dram_tensor("ones2", [128, HD], F32R, kind="ExternalInput").ap()
    out = nc.dram_tensor("out", [BS, H], F32, kind="ExternalOutput").ap()

    NK = H // 128          # 8 contraction chunks for qkv
    NN = BS // 512         # 8 token blocks of 512
    NQB = S // 512         # 4 query blocks per batch
    NKC = S // 128         # 16 key chunks per batch

    with tile.TileContext(nc) as tc, nc.allow_low_precision(reason="fp32r"):
        from contextlib import ExitStack
        with ExitStack() as ctx:
            consts = ctx.enter_context(tc.tile_pool(name="consts", bufs=1))
            mixed = ctx.enter_context(tc.tile_pool(name="mixed", bufs=1))
            ctxp = ctx.enter_context(tc.tile_pool(name="ctxp", bufs=1))
            xtp = ctx.enter_context(tc.tile_pool(name="xtp", bufs=5))
            vsb = ctx.enter_context(tc.tile_pool(name="vsb", bufs=2))
            expp = ctx.enter_context(tc.tile_pool(name="expp", bufs=4))
            sums = ctx.enter_context(tc.tile_pool(name="sums", bufs=2))
            ctxf_p = ctx.enter_context(tc.tile_pool(name="ctxf", bufs=2))
            rbp = ctx.enter_context(tc.tile_pool(name="rbp", bufs=2))
            outs = ctx.enter_context(tc.tile_pool(name="outs", bufs=4))
            ps_sc = ctx.enter_context(tc.tile_pool(name="ps_sc", bufs=2, space="PSUM"))
            ps_ac = ctx.enter_context(tc.tile_pool(name="ps_ac", bufs=2, space="PSUM"))
            ps_ms = ctx.enter_context(tc.tile_pool(name="ps_ms", bufs=2, space="PSUM"))

            # ---- constants ----
            w1big = consts.tile([128, NK, ROWS_PER_CORE], F32R, name="w1big")
            w1r = w1t.rearrange("(k p) r -> p k r", p=128)
            nc.sync.dma_start(w1big[:, 0:NK // 2, :], w1r[:, 0:NK // 2, :])
            nc.sync.dma_start(w1big[:, NK // 2:NK, :], w1r[:, NK // 2:NK, :])
            b1sb = consts.tile([128, 3], F32, name="b1")
            nc.sync.dma_start(b1sb[:], b1)
            eye2sb = consts.tile([128, HD], F32R, name="eye2")
            nc.sync.dma_start(eye2sb[:], eye2)
            ones2sb = consts.tile([128, HD], F32R, name="ones2")
            nc.sync.dma_start(ones2sb[:], ones2)
            w2sb = consts.tile([128, H], F32R, name="w2pack")
            nc.sync.dma_start(w2sb[0:HD, :], w2t0)
            nc.sync.dma_start(w2sb[HD:128, :], w2t1)

            # ---- phase A building blocks ----
            qt = mixed.tile([128, BS], F32R, name="qt")
            kt = mixed.tile([128, BS], F32R, name="kt")
            vt = mixed.tile([128, BS], F32R, name="vt")
            mix_dst = [qt, kt, vt]
            KG = 4  # k-chunks per xt DMA

            def emit_qkv_nblock(n):
                """mixedT[:, n*512:(n+1)*512] = W1 @ XT block (+bias).
                m-outer / k-inner: one PSUM slot at a time, PE K-contiguous."""
                xts = []
                for kg in range(NK // KG):
                    xt_t = xtp.tile([128, KG, 512], F32R, name="xt")
                    nc.sync.dma_start(
                        xt_t[:],
                        xt[kg * KG * 128:(kg + 1) * KG * 128,
                           n * 512:(n + 1) * 512].rearrange(
                               "(c p) f -> p c f", p=128))
                    xts.append(xt_t)
                for m in range(3):
                    ps = ps_ac.tile([128, 512], F32, name=f"qkv{m}", tag="acc")
                    for k in range(NK):
                        nc.tensor.matmul(
                            ps[:],
                            w1big[:, k, m * 128:(m + 1) * 128],
                            xts[k // KG][:, k % KG, :],
                            start=(k == 0), stop=(k == NK - 1))
                    nc.scalar.activation(
                        mix_dst[m][:, n * 512:(n + 1) * 512], ps[:],
                        Act.Identity, bias=b1sb[:, m:m + 1])

            def emit_vprep(b):
                vbig = {}
                for j in range(2):
                    vb = vsb.tile([128, NKC * (HD + 1)], F32R, name=f"vbig{j}")
                    ones_view = vb[:].rearrange(
                        "p (c w) -> p c w", w=HD + 1)[:, :, HD:HD + 1]
                    nc.vector.tensor_copy(ones_view, ones2sb[:, 0:NKC])
                    for kc in range(NKC):
                        pt = ps_ms.tile([128, HD], F32R, name="vtr", tag="misc")
                        nc.tensor.transpose(
                            pt[:],
                            vt[64 * j:64 * j + 64,
                               b * S + kc * 128:b * S + (kc + 1) * 128],
                            eye2sb[64 * j:64 * j + 64, :])
                        nc.vector.tensor_copy(
                            vb[:, kc * (HD + 1):kc * (HD + 1) + HD], pt[:])
                    vbig[j] = vb
                return vbig

            def emit_attention_qb(b, qb, vbig, cts):
                ctxps = {j: ps_ac.tile([HD + 1, 512], F32, name=f"ctxps{j}",
                                       tag="acc")
                         for j in range(2)}
                for kc in range(NKC):
                    sp2 = ps_sc.tile([128, 1024], F32, name="scores")
                    for j in range(2):
                        nc.tensor.matmul(
                            sp2[:, j * 512:(j + 1) * 512],
                            kt[64 * j:64 * j + 64,
                               b * S + kc * 128:b * S + (kc + 1) * 128],
                            qt[64 * j:64 * j + 64,
                               b * S + qb * 512:b * S + (qb + 1) * 512],
                            start=True, stop=True)
                    et2 = expp.tile([128, 1024], F32R, name="exp")
                    nc.scalar.activation(et2[:], sp2[:], Act.Exp, scale=0.125)
                    for j in range(2):
                        nc.tensor.matmul(
                            ctxps[j][:],
                            vbig[j][:, kc * (HD + 1):(kc + 1) * (HD + 1)],
                            et2[:, j * 512:(j + 1) * 512],
                            start=(kc == 0), stop=(kc == NKC - 1))
                for j in range(2):
                    # free the accumulator bank fast: stage PSUM -> SBUF, then
                    # finish the normalization from SBUF off the critical path
                    ctxf = ctxf_p.tile([HD, 512], F32, name="ctxf")
                    nc.vector.tensor_copy(ctxf[:], ctxps[j][0:HD, :])
                    ss = sums.tile([1, 512], F32, name="sums")
                    nc.vector.tensor_copy(ss[0:1, :], ctxps[j][HD:HD + 1, :])
                    rbb = rbp.tile([HD, 512], F32, name="rbb")
                    nc.gpsimd.partition_broadcast(rbb[:], ss[0:1, :])
                    rb = rbp.tile([HD, 512], F32, name="rb")
                    nc.vector.reciprocal_approx_fast(rb[:], rbb[:])
                    nc.vector.tensor_mul(
                        cts[64 * j:64 * (j + 1), qb * 512:(qb + 1) * 512],
                        ctxf[:], rb[:])

            def emit_dense_qb(b, qb, cts):
                """Dense partial for the 512-token block qb (4 t-chunks)."""
                for tp in range(2):
                    ob = outs.tile([128, 2, H], F32, name="ostage")
                    for tc in range(2):
                        t = qb * 4 + tp * 2 + tc
                        for nb in range(2):
                            dp = ps_ms.tile([128, 512], F32, name="dense",
                                            tag="misc")
                            nc.tensor.matmul(
                                dp[:], cts[:, t * 128:(t + 1) * 128],
                                w2sb[:, nb * 512:(nb + 1) * 512],
                                start=True, stop=True)
                            nc.vector.tensor_copy(
                                ob[:, tc, nb * 512:(nb + 1) * 512], dp[:])
                    row0 = b * S + (qb * 2 + tp) * 256
                    nc.sync.dma_start(
                        out[row0:row0 + 256, :].rearrange(
                            "(c p) f -> p c f", p=128),
                        ob[:])

            # ---- emission schedule ----
            # Serial qkv projection (DMA-paced; PE slack absorbs the v
            # transposes), then per-batch attention with dense interleaved
            # per query block so the epilogue never piles up at the end.
            for n in range(NN // 2):
                emit_qkv_nblock(n)
            vbig0 = emit_vprep(0)
            for n in range(NN // 2, NN):
                emit_qkv_nblock(n)
            vbig1 = emit_vprep(1)
            cts0 = ctxp.tile([128, S], F32R, name="ctx_0")
            for qb in range(NQB):
                emit_attention_qb(0, qb, vbig0, cts0)
                emit_dense_qb(0, qb, cts0)
            cts1 = ctxp.tile([128, S], F32R, name="ctx_1")
            for qb in range(NQB):
                emit_attention_qb(1, qb, vbig1, cts1)
                emit_dense_qb(1, qb, cts1)
    nc.compile()
    return nc


def _prepare_inputs(hidden_states, qkv_w, qkv_b, dense_w):
    """Build per-core input maps (all host-side slicing/transposition)."""
    x = np.ascontiguousarray(hidden_states, dtype=np.float32).reshape(BS, H)
    xt = _round_fp32r(np.ascontiguousarray(x.T))
    eye2 = np.concatenate([np.eye(HD, dtype=np.float32)] * 2, axis=0)
    ones2 = np.ones((128, HD), dtype=np.float32)
    in_maps = []
    for c in range(NCORES):
        base = c * ROWS_PER_CORE
        # per-head row groups within this core's 384 rows: h0 {q,k,v}, h1 {q,k,v}
        rows = {}
        for m in range(3):  # 0=q 1=k 2=v
            rows[m] = np.r_[base + m * HD:base + (m + 1) * HD,
                            base + 192 + m * HD:base + 192 + (m + 1) * HD]
        perm = np.concatenate([rows[0], rows[1], rows[2]])
        w1t = _round_fp32r(np.ascontiguousarray(qkv_w[perm, :].T))   # [H, 384]
        b1 = np.ascontiguousarray(
            np.stack([qkv_b[rows[m]] for m in range(3)], axis=1),
            dtype=np.float32)                                        # [128, 3]
        w2t0 = _round_fp32r(np.ascontiguousarray(
            dense_w[:, c * DPC:c * DPC + HD].T))                     # [64, 1024]
        w2t1 = _round_fp32r(np.ascontiguousarray(
            dense_w[:, c * DPC + HD:(c + 1) * DPC].T))
        in_maps.append({
            "xt": xt, "w1t": w1t, "b1": b1,
            "w2t0": w2t0, "w2t1": w2t1,
            "eye2": eye2, "ones2": ones2,
        })
    return in_maps


def _reference_numpy(hidden_states, attention_mask, qkv_w, qkv_b, dense_w, dense_b):
    """Exact fallback for non-all-ones masks (never hit with spec inputs)."""
    x = np.asarray(hidden_states, dtype=np.float64)
    mask = np.asarray(attention_mask, dtype=np.float64)
    mixed = x @ np.asarray(qkv_w, np.float64).T + np.asarray(qkv_b, np.float64)
    mixed = mixed.reshape(B, S, NH, 3 * HD).transpose(0, 2, 1, 3)
    q, k, v = np.split(mixed, 3, axis=-1)
    scores = np.einsum("bhqd,bhkd->bhqk", q, k) / np.sqrt(HD)
    scores = scores * mask - 10000.0 * (1.0 - mask)
    scores -= scores.max(axis=-1, keepdims=True)
    probs = np.exp(scores)
    probs /= probs.sum(axis=-1, keepdims=True)
    cx = np.einsum("bhqk,bhkd->bhqd", probs, v)
    cx = cx.transpose(0, 2, 1, 3).reshape(B, S, H)
    o = cx @ np.asarray(dense_w, np.float64).T + np.asarray(dense_b, np.float64)
    return o.astype(np.float32)


def _run(inputs, trace=False):
    from concourse.bass_utils import run_bass_kernel_spmd
    if "nc" not in _CACHE:
        _CACHE["nc"] = _build_program()
    nc = _CACHE["nc"]
    in_maps = _prepare_inputs(inputs["hidden_states"], inputs["qkv_w"],
                              inputs["qkv_b"], inputs["dense_w"])
    res = run_bass_kernel_spmd(nc, in_maps, core_ids=list(range(NCORES)),
                               trace=trace)
    partials = np.stack([r["out"] for r in res.results], axis=0)
    full = partials.sum(axis=0, dtype=np.float64)
    full += np.asarray(inputs["dense_b"], dtype=np.float64)
    return full.astype(np.float32).reshape(B, S, H), res


def kernel(hidden_states, attention_mask, qkv_w, qkv_b, dense_w, dense_b):
    hidden_states = np.asarray(hidden_states)
    attention_mask = np.asarray(attention_mask)
    qkv_w = np.asarray(qkv_w)
    qkv_b = np.asarray(qkv_b)
    dense_w = np.asarray(dense_w)
    dense_b = np.asarray(dense_b)
    if not np.all(attention_mask == 1.0):
        return _reference_numpy(hidden_states, attention_mask, qkv_w, qkv_b,
                                dense_w, dense_b)
    out, _ = _run({
        "hidden_states": hidden_states, "qkv_w": qkv_w, "qkv_b": qkv_b,
        "dense_w": dense_w, "dense_b": dense_b,
    }, trace=bool(int(os.environ.get("KERNEL_TRACE", "0"))))
    return out


# revision 18
# speedup vs baseline: 1.0443x; 1.0443x over previous
"""Trainium2 Bass kernel for nn_Attention: 16-head attention layer, B=2, S=2048, H=1024.

Strategy (Megatron-style tensor parallel over heads, 8 cores x 2 heads):
  - Host transposes hidden_states once (XT [H, B*S]) and pre-rounds all matmul
    inputs to fp32r (TF32-like: 11-bit mantissa) so every matmul runs at the
    full 1-cycle/row PE rate with fp32 accumulation.
  - Each core computes its 2 heads' q/k/v via XT @ its W slice (transposed
    layout), attention with softmax folded as exp -> matmul-rowsum -> late
    normalization, then a partial dense projection over its 128 ctx columns.
  - Host sums the 8 partial dense outputs and adds dense_b.

All computed on device except the final 8-way partial reduction (done at
gather time on host, per the Megatron all-reduce-after-dense recipe).
"""
import os
import numpy as np

B, S, H, NH = 2, 2048, 1024, 16
HD = H // NH            # 64
BS = B * S              # 4096
NCORES = 8
ROWS_PER_CORE = 3 * HD * 2   # 384 qkv rows per core
DPC = 2 * HD                 # 128 ctx/dense columns per core

_CACHE = {}


def _round_fp32r(x):
    bits = np.ascontiguousarray(x, dtype=np.float32).view(np.uint32)
    lsb = (bits >> np.uint32(12)) & np.uint32(1)
    return ((bits + np.uint32(0x7FF) + lsb) & np.uint32(0xFFFFF000)).view(np.float32)


def _build_program():
    import concourse.mybir as mybir
    import concourse.tile as tile
    from concourse import bacc

    F32 = mybir.dt.float32
    F32R = mybir.dt.float32r
    Act = mybir.ActivationFunctionType

    nc = bacc.Bacc("TRN2", target_bir_lowering=False, debug=False,
                   num_devices=NCORES)
    xt = nc.dram_tensor("xt", [H, BS], F32R, kind="ExternalInput").ap()
    w1t = nc.dram_tensor("w1t", [H, ROWS_PER_CORE], F32R, kind="ExternalInput").ap()
    b1 = nc.dram_tensor("b1", [128, 3], F32, kind="ExternalInput").ap()
    w2t0 = nc.dram_tensor("w2t0", [HD, H], F32R, kind="ExternalInput").ap()
    w2t1 = nc.dram_tensor("w2t1", [HD, H], F32R, kind="ExternalInput").ap()
    eye2 = nc.dram_tensor("eye2", [128, HD], F32R, kind="ExternalInput").ap()
    ones2 = nc.dram_tensor("ones2", [128, HD], F32R, kind="ExternalInput").ap()
    out = nc.dram_tensor("out", [BS, H], F32, kind="ExternalOutput").ap()

    NK = H // 128          # 8 contraction chunks for qkv
    NN = BS // 512         # 8 token blocks of 512
    NQB = S // 512         # 4 query blocks per batch
    NKC = S // 128         # 16 key chunks per batch

    with tile.TileContext(nc) as tc, nc.allow_low_precision(reason="fp32r"):
        from contextlib import ExitStack
        with ExitStack() as ctx:
            consts = ctx.enter_context(tc.tile_pool(name="consts", bufs=1))
            mixed = ctx.enter_context(tc.tile_pool(name="mixed", bufs=1))
            ctxp = ctx.enter_context(tc.tile_pool(name="ctxp", bufs=1))
            xtp = ctx.enter_context(tc.tile_pool(name="xtp", bufs=5))
            vsb = ctx.enter_context(tc.tile_pool(name="vsb", bufs=2))
            expp = ctx.enter_context(tc.tile_pool(name="expp", bufs=4))
            sums = ctx.enter_context(tc.tile_pool(name="sums", bufs=2))
            ctxf_p = ctx.enter_context(tc.tile_pool(name="ctxf", bufs=2))
            rbp = ctx.enter_context(tc.tile_pool(name="rbp", bufs=2))
            outs = ctx.enter_context(tc.tile_pool(name="outs", bufs=4))
            ps_sc = ctx.enter_context(tc.tile_pool(name="ps_sc", bufs=2, space="PSUM"))
            ps_ac = ctx.enter_context(tc.tile_pool(name="ps_ac", bufs=2, space="PSUM"))
            ps_ms = ctx.enter_context(tc.tile_pool(name="ps_ms", bufs=2, space="PSUM"))

            # ---- constants ----
            w1big = consts.tile([128, NK, ROWS_PER_CORE], F32R, name="w1big")
            w1r = w1t.rearrange("(k p) r -> p k r", p=128)
            nc.sync.dma_start(w1big[:, 0:1, :], w1r[:, 0:1, :])
            nc.sync.dma_start(w1big[:, 1:NK // 2, :], w1r[:, 1:NK // 2, :])
            nc.sync.dma_start(w1big[:, NK // 2:NK, :], w1r[:, NK // 2:NK, :])
            b1sb = consts.tile([128, 3], F32, name="b1")
            nc.sync.dma_start(b1sb[:], b1)
            warm = consts.tile([1, 1], F32, name="warm")
            nc.scalar.activation(warm[0:1, 0:1], b1sb[0:1, 0:1], Act.Exp)
            eye2sb = consts.tile([128, HD], F32R, name="eye2")
            nc.sync.dma_start(eye2sb[:], eye2)
            ones2sb = consts.tile([128, HD], F32R, name="ones2")
            nc.sync.dma_start(ones2sb[:], ones2)
            w2sb = consts.tile([128, H], F32R, name="w2pack")
            nc.sync.dma_start(w2sb[0:HD, :], w2t0)
            nc.sync.dma_start(w2sb[HD:128, :], w2t1)

            # ---- phase A building blocks ----
            qt = mixed.tile([128, BS], F32R, name="qt")
            kt = mixed.tile([128, BS], F32R, name="kt")
            vt = mixed.tile([128, BS], F32R, name="vt")
            mix_dst = [qt, kt, vt]
            KG = 4  # k-chunks per xt DMA

            def emit_qkv_nblock(n, fine=False):
                """mixedT[:, n*512:(n+1)*512] = W1 @ XT block (+bias).
                m-outer / k-inner: one PSUM slot at a time, PE K-contiguous.
                fine=True splits the loads per k-chunk so the first matmul
                starts as soon as 256KB has landed (kernel warmup)."""
                xts = []
                for kg in range(NK // KG):
                    xt_t = xtp.tile([128, KG, 512], F32R, name="xt")
                    if fine:
                        for c in range(KG):
                            k = kg * KG + c
                            nc.sync.dma_start(
                                xt_t[:, c, :],
                                xt[k * 128:(k + 1) * 128,
                                   n * 512:(n + 1) * 512])
                    else:
                        nc.sync.dma_start(
                            xt_t[:],
                            xt[kg * KG * 128:(kg + 1) * KG * 128,
                               n * 512:(n + 1) * 512].rearrange(
                                   "(c p) f -> p c f", p=128))
                    xts.append(xt_t)
                for m in range(3):
                    ps = ps_ac.tile([128, 512], F32, name=f"qkv{m}", tag="acc")
                    for k in range(NK):
                        nc.tensor.matmul(
                            ps[:],
                            w1big[:, k, m * 128:(m + 1) * 128],
                            xts[k // KG][:, k % KG, :],
                            start=(k == 0), stop=(k == NK - 1))
                    nc.scalar.activation(
                        mix_dst[m][:, n * 512:(n + 1) * 512], ps[:],
                        Act.Identity, bias=b1sb[:, m:m + 1])

            def emit_vprep(b):
                vbig = {}
                for j in range(2):
                    vb = vsb.tile([128, NKC * (HD + 1)], F32R, name=f"vbig{j}")
                    ones_view = vb[:].rearrange(
                        "p (c w) -> p c w", w=HD + 1)[:, :, HD:HD + 1]
                    nc.vector.tensor_copy(ones_view, ones2sb[:, 0:NKC])
                    for kc in range(NKC):
                        pt = ps_ms.tile([128, HD], F32R, name="vtr", tag="misc")
                        nc.tensor.transpose(
                            pt[:],
                            vt[64 * j:64 * j + 64,
                               b * S + kc * 128:b * S + (kc + 1) * 128],
                            eye2sb[64 * j:64 * j + 64, :])
                        nc.vector.tensor_copy(
                            vb[:, kc * (HD + 1):kc * (HD + 1) + HD], pt[:])
                    vbig[j] = vb
                return vbig

            def emit_attention_qb(b, qb, vbig, cts):
                ctxps = {j: ps_ac.tile([HD + 1, 512], F32, name=f"ctxps{j}",
                                       tag="acc")
                         for j in range(2)}
                for kc in range(NKC):
                    sp2 = ps_sc.tile([128, 1024], F32, name="scores")
                    for j in range(2):
                        nc.tensor.matmul(
                            sp2[:, j * 512:(j + 1) * 512],
                            kt[64 * j:64 * j + 64,
                               b * S + kc * 128:b * S + (kc + 1) * 128],
                            qt[64 * j:64 * j + 64,
                               b * S + qb * 512:b * S + (qb + 1) * 512],
                            start=True, stop=True)
                    et2 = expp.tile([128, 1024], F32R, name="exp")
                    nc.scalar.activation(et2[:], sp2[:], Act.Exp, scale=0.125)
                    for j in range(2):
                        nc.tensor.matmul(
                            ctxps[j][:],
                            vbig[j][:, kc * (HD + 1):(kc + 1) * (HD + 1)],
                            et2[:, j * 512:(j + 1) * 512],
                            start=(kc == 0), stop=(kc == NKC - 1))
                for j in range(2):
                    # free the accumulator bank after a single copy; the rest
                    # of the normalization runs from SBUF off the critical path
                    ctxf = ctxf_p.tile([HD + 1, 512], F32, name="ctxf")
                    nc.vector.tensor_copy(ctxf[:], ctxps[j][:])
                    ss = sums.tile([1, 512], F32, name="sums")
                    nc.vector.tensor_copy(ss[0:1, :], ctxf[HD:HD + 1, :])
                    rbb = rbp.tile([HD, 512], F32, name="rbb")
                    nc.gpsimd.partition_broadcast(rbb[:], ss[0:1, :])
                    rb = rbp.tile([HD, 512], F32, name="rb")
                    nc.vector.reciprocal_approx_fast(rb[:], rbb[:])
                    nc.vector.tensor_mul(
                        cts[64 * j:64 * (j + 1), qb * 512:(qb + 1) * 512],
                        ctxf[0:HD, :], rb[:])

            def emit_dense_qb(b, qb, cts):
                """Dense partial for the 512-token block qb (4 t-chunks)."""
                for t4 in range(4):
                    t = qb * 4 + t4
                    ob = outs.tile([128, H], F32, name="ostage")
                    for nb in range(2):
                        dp = ps_ms.tile([128, 512], F32, name="dense",
                                        tag="misc")
                        nc.tensor.matmul(
                            dp[:], cts[:, t * 128:(t + 1) * 128],
                            w2sb[:, nb * 512:(nb + 1) * 512],
                            start=True, stop=True)
                        nc.vector.tensor_copy(
                            ob[:, nb * 512:(nb + 1) * 512], dp[:])
                    row0 = b * S + t * 128
                    nc.sync.dma_start(out[row0:row0 + 128, :], ob[:])

            # ---- emission schedule ----
            # Serial qkv projection (DMA-paced; PE slack absorbs the v
            # transposes), then per-batch attention with dense interleaved
            # per query block so the epilogue never piles up at the end.
            for n in range(NN // 2):
                emit_qkv_nblock(n, fine=(n == 0))
            vbig0 = emit_vprep(0)
            cts0 = ctxp.tile([128, S], F32R, name="ctx_0")
            emit_attention_qb(0, 0, vbig0, cts0)
            for n in range(NN // 2, NN):
                emit_qkv_nblock(n)
            vbig1 = emit_vprep(1)
            emit_dense_qb(0, 0, cts0)
            for qb in range(1, NQB):
                emit_attention_qb(0, qb, vbig0, cts0)
                emit_dense_qb(0, qb, cts0)
            cts1 = ctxp.tile([128, S], F32R, name="ctx_1")
            for qb in range(NQB):
                emit_attention_qb(1, qb, vbig1, cts1)
                emit_dense_qb(1, qb, cts1)
    nc.compile()
    return nc


def _prepare_inputs(hidden_states, qkv_w, qkv_b, dense_w):
    """Build per-core input maps (all host-side slicing/transposition)."""
    x = np.ascontiguousarray(hidden_states, dtype=np.float32).reshape(BS, H)
    xt = _round_fp32r(np.ascontiguousarray(x.T))
    eye2 = np.concatenate([np.eye(HD, dtype=np.float32)] * 2, axis=0)
    ones2 = np.ones((128, HD), dtype=np.float32)
    in_maps = []
    for c in range(NCORES):
        base = c * ROWS_PER_CORE
        # per-head row groups within this core's 384 rows: h0 {q,k,v}, h1 {q,k,v}
        rows = {}
        for m in range(3):  # 0=q 1=k 2=v
            rows[m] = np.r_[base + m * HD:base + (m + 1) * HD,
                            base + 192 + m * HD:base + 192 + (m + 1) * HD]
        perm = np.concatenate([rows[0], rows[1], rows[2]])
        w1t = _round_fp32r(np.ascontiguousarray(qkv_w[perm, :].T))   # [H, 384]
        b1 = np.ascontiguousarray(
            np.stack([qkv_b[rows[m]] for m in range(3)], axis=1),
            dtype=np.float32)                                        # [128, 3]
        w2t0 = _round_fp32r(np.ascontiguousarray(
            dense_w[:, c * DPC:c * DPC + HD].T))                     # [64, 1024]
        w2t1 = _round_fp32r(np.ascontiguousarray(
            dense_w[:, c * DPC + HD:(c + 1) * DPC].T))
        in_maps.append({
            "xt": xt, "w1t": w1t, "b1": b1,
            "w2t0": w2t0, "w2t1": w2t1,
            "eye2": eye2, "ones2": ones2,
        })
    return in_maps


def _reference_numpy(hidden_states, attention_mask, qkv_w, qkv_b, dense_w, dense_b):
    """Exact fallback for non-all-ones masks (never hit with spec inputs)."""
    x = np.asarray(hidden_states, dtype=np.float64)
    mask = np.asarray(attention_mask, dtype=np.float64)
    mixed = x @ np.asarray(qkv_w, np.float64).T + np.asarray(qkv_b, np.float64)
    mixed = mixed.reshape(B, S, NH, 3 * HD).transpose(0, 2, 1, 3)
    q, k, v = np.split(mixed, 3, axis=-1)
    scores = np.einsum("bhqd,bhkd->bhqk", q, k) / np.sqrt(HD)
    scores = scores * mask - 10000.0 * (1.0 - mask)
    scores -= scores.max(axis=-1, keepdims=True)
    probs = np.exp(scores)
    probs /= probs.sum(axis=-1, keepdims=True)
    cx = np.einsum("bhqk,bhkd->bhqd", probs, v)
    cx = cx.transpose(0, 2, 1, 3).reshape(B, S, H)
    o = cx @ np.asarray(dense_w, np.float64).T + np.asarray(dense_b, np.float64)
    return o.astype(np.float32)


def _run(inputs, trace=False):
    from concourse.bass_utils import run_bass_kernel_spmd
    if "nc" not in _CACHE:
        _CACHE["nc"] = _build_program()
    nc = _CACHE["nc"]
    in_maps = _prepare_inputs(inputs["hidden_states"], inputs["qkv_w"],
                              inputs["qkv_b"], inputs["dense_w"])
    res = run_bass_kernel_spmd(nc, in_maps, core_ids=list(range(NCORES)),
                               trace=trace)
    partials = np.stack([r["out"] for r in res.results], axis=0)
    full = partials.sum(axis=0, dtype=np.float64)
    full += np.asarray(inputs["dense_b"], dtype=np.float64)
    return full.astype(np.float32).reshape(B, S, H), res


def kernel(hidden_states, attention_mask, qkv_w, qkv_b, dense_w, dense_b):
    hidden_states = np.asarray(hidden_states)
    attention_mask = np.asarray(attention_mask)
    qkv_w = np.asarray(qkv_w)
    qkv_b = np.asarray(qkv_b)
    dense_w = np.asarray(dense_w)
    dense_b = np.asarray(dense_b)
    if not np.all(attention_mask == 1.0):
        return _reference_numpy(hidden_states, attention_mask, qkv_w, qkv_b,
                                dense_w, dense_b)
    out, _ = _run({
        "hidden_states": hidden_states, "qkv_w": qkv_w, "qkv_b": qkv_b,
        "dense_w": dense_w, "dense_b": dense_b,
    }, trace=bool(int(os.environ.get("KERNEL_TRACE", "0"))))
    return out


# revision 25
# speedup vs baseline: 1.0686x; 1.0233x over previous
"""Trainium2 Bass kernel for nn_Attention: 16-head attention layer, B=2, S=2048, H=1024.

Strategy (Megatron-style tensor parallel over heads, 8 cores x 2 heads):
  - Host transposes hidden_states once (XT [H, B*S]) and pre-rounds all matmul
    inputs to fp32r (TF32-like: 11-bit mantissa) so every matmul runs at the
    full 1-cycle/row PE rate with fp32 accumulation.
  - Each core computes its 2 heads' q/k/v via XT @ its W slice (transposed
    layout), attention with softmax folded as exp -> matmul-rowsum -> late
    normalization, then a partial dense projection over its 128 ctx columns.
  - Host sums the 8 partial dense outputs and adds dense_b.

All computed on device except the final 8-way partial reduction (done at
gather time on host, per the Megatron all-reduce-after-dense recipe).
"""
import os
import numpy as np

B, S, H, NH = 2, 2048, 1024, 16
HD = H // NH            # 64
BS = B * S              # 4096
NCORES = 8
ROWS_PER_CORE = 3 * HD * 2   # 384 qkv rows per core
DPC = 2 * HD                 # 128 ctx/dense columns per core

_CACHE = {}


def _round_fp32r(x):
    bits = np.ascontiguousarray(x, dtype=np.float32).view(np.uint32)
    lsb = (bits >> np.uint32(12)) & np.uint32(1)
    return ((bits + np.uint32(0x7FF) + lsb) & np.uint32(0xFFFFF000)).view(np.float32)


def _build_program():
    import concourse.mybir as mybir
    import concourse.tile as tile
    from concourse import bacc

    F32 = mybir.dt.float32
    F32R = mybir.dt.float32r
    Act = mybir.ActivationFunctionType

    nc = bacc.Bacc("TRN2", target_bir_lowering=False, debug=False,
                   num_devices=NCORES)
    xt = nc.dram_tensor("xt", [H, BS], F32R, kind="ExternalInput").ap()
    w1t = nc.dram_tensor("w1t", [H, ROWS_PER_CORE], F32R, kind="ExternalInput").ap()
    b1 = nc.dram_tensor("b1", [128, 3], F32, kind="ExternalInput").ap()
    w2t0 = nc.dram_tensor("w2t0", [HD, H], F32R, kind="ExternalInput").ap()
    w2t1 = nc.dram_tensor("w2t1", [HD, H], F32R, kind="ExternalInput").ap()
    eye2 = nc.dram_tensor("eye2", [128, HD], F32R, kind="ExternalInput").ap()
    ones2 = nc.dram_tensor("ones2", [128, HD], F32R, kind="ExternalInput").ap()
    out = nc.dram_tensor("out", [BS, H], F32, kind="ExternalOutput").ap()

    NK = H // 128          # 8 contraction chunks for qkv
    NN = BS // 512         # 8 token blocks of 512
    NQB = S // 512         # 4 query blocks per batch
    NKC = S // 128         # 16 key chunks per batch

    with tile.TileContext(nc) as tc, nc.allow_low_precision(reason="fp32r"):
        from contextlib import ExitStack
        with ExitStack() as ctx:
            consts = ctx.enter_context(tc.tile_pool(name="consts", bufs=1))
            mixed = ctx.enter_context(tc.tile_pool(name="mixed", bufs=1))
            ctxp = ctx.enter_context(tc.tile_pool(name="ctxp", bufs=1))
            xtp = ctx.enter_context(tc.tile_pool(name="xtp", bufs=5))
            vsb = ctx.enter_context(tc.tile_pool(name="vsb", bufs=2))
            expp = ctx.enter_context(tc.tile_pool(name="expp", bufs=4))
            sums = ctx.enter_context(tc.tile_pool(name="sums", bufs=2))
            ctxf_p = ctx.enter_context(tc.tile_pool(name="ctxf", bufs=2))
            rbp = ctx.enter_context(tc.tile_pool(name="rbp", bufs=2))
            outs = ctx.enter_context(tc.tile_pool(name="outs", bufs=4))
            ps_sc = ctx.enter_context(tc.tile_pool(name="ps_sc", bufs=2, space="PSUM"))
            ps_ac = ctx.enter_context(tc.tile_pool(name="ps_ac", bufs=2, space="PSUM"))
            ps_ms = ctx.enter_context(tc.tile_pool(name="ps_ms", bufs=2, space="PSUM"))

            # ---- constants ----
            w1big = consts.tile([128, NK, ROWS_PER_CORE], F32R, name="w1big")
            w1r = w1t.rearrange("(k p) r -> p k r", p=128)
            nc.sync.dma_start(w1big[:, 0:1, :], w1r[:, 0:1, :])
            nc.sync.dma_start(w1big[:, 1:NK // 2, :], w1r[:, 1:NK // 2, :])
            nc.sync.dma_start(w1big[:, NK // 2:NK, :], w1r[:, NK // 2:NK, :])
            b1sb = consts.tile([128, 3], F32, name="b1")
            nc.sync.dma_start(b1sb[:], b1)
            warm = consts.tile([1, 1], F32, name="warm")
            nc.scalar.activation(warm[0:1, 0:1], b1sb[0:1, 0:1], Act.Exp)
            eye2sb = consts.tile([128, HD], F32R, name="eye2")
            nc.sync.dma_start(eye2sb[:], eye2)
            ones2sb = consts.tile([128, HD], F32R, name="ones2")
            nc.sync.dma_start(ones2sb[:], ones2)
            w2sb = consts.tile([128, H], F32R, name="w2pack")
            nc.sync.dma_start(w2sb[0:HD, :], w2t0)
            nc.sync.dma_start(w2sb[HD:128, :], w2t1)

            # ---- phase A building blocks ----
            qt = mixed.tile([128, BS], F32R, name="qt")
            kt = mixed.tile([128, BS], F32R, name="kt")
            vt = mixed.tile([128, BS], F32R, name="vt")
            mix_dst = [qt, kt, vt]
            KG = 4  # k-chunks per xt DMA

            def emit_qkv_nblock(n, fine=False):
                """mixedT[:, n*512:(n+1)*512] = W1 @ XT block (+bias).
                m-outer / k-inner: one PSUM slot at a time, PE K-contiguous.
                fine=True splits the loads per k-chunk so the first matmul
                starts as soon as 256KB has landed (kernel warmup)."""
                xts = []
                for kg in range(NK // KG):
                    xt_t = xtp.tile([128, KG, 512], F32R, name="xt")
                    if fine:
                        for c in range(KG):
                            k = kg * KG + c
                            nc.sync.dma_start(
                                xt_t[:, c, :],
                                xt[k * 128:(k + 1) * 128,
                                   n * 512:(n + 1) * 512])
                    else:
                        nc.sync.dma_start(
                            xt_t[:],
                            xt[kg * KG * 128:(kg + 1) * KG * 128,
                               n * 512:(n + 1) * 512].rearrange(
                                   "(c p) f -> p c f", p=128))
                    xts.append(xt_t)
                for m in range(3):
                    ps = ps_ac.tile([128, 512], F32, name=f"qkv{m}", tag="acc")
                    for k in range(NK):
                        nc.tensor.matmul(
                            ps[:],
                            w1big[:, k, m * 128:(m + 1) * 128],
                            xts[k // KG][:, k % KG, :],
                            start=(k == 0), stop=(k == NK - 1))
                    nc.scalar.activation(
                        mix_dst[m][:, n * 512:(n + 1) * 512], ps[:],
                        Act.Identity, bias=b1sb[:, m:m + 1])

            def emit_vprep(b):
                vbig = {}
                for j in range(2):
                    vb = vsb.tile([128, NKC * (HD + 1)], F32R, name=f"vbig{j}")
                    ones_view = vb[:].rearrange(
                        "p (c w) -> p c w", w=HD + 1)[:, :, HD:HD + 1]
                    nc.vector.tensor_copy(ones_view, ones2sb[:, 0:NKC])
                    for kc in range(NKC):
                        pt = ps_ms.tile([128, HD], F32R, name="vtr", tag="misc")
                        nc.tensor.transpose(
                            pt[:],
                            vt[64 * j:64 * j + 64,
                               b * S + kc * 128:b * S + (kc + 1) * 128],
                            eye2sb[64 * j:64 * j + 64, :])
                        nc.vector.tensor_copy(
                            vb[:, kc * (HD + 1):kc * (HD + 1) + HD], pt[:])
                    vbig[j] = vb
                return vbig

            def emit_attention_kc(b, qb, vbig):
                ctxps = {j: ps_ac.tile([HD + 1, 512], F32, name=f"ctxps{j}",
                                       tag="acc")
                         for j in range(2)}
                for kc in range(NKC):
                    sp2 = ps_sc.tile([128, 1024], F32, name="scores")
                    for j in range(2):
                        nc.tensor.matmul(
                            sp2[:, j * 512:(j + 1) * 512],
                            kt[64 * j:64 * j + 64,
                               b * S + kc * 128:b * S + (kc + 1) * 128],
                            qt[64 * j:64 * j + 64,
                               b * S + qb * 512:b * S + (qb + 1) * 512],
                            start=True, stop=True)
                    et2 = expp.tile([128, 1024], F32R, name="exp")
                    nc.scalar.activation(et2[:], sp2[:], Act.Exp, scale=0.125)
                    for j in range(2):
                        nc.tensor.matmul(
                            ctxps[j][:],
                            vbig[j][:, kc * (HD + 1):(kc + 1) * (HD + 1)],
                            et2[:, j * 512:(j + 1) * 512],
                            start=(kc == 0), stop=(kc == NKC - 1))
                return ctxps

            def emit_norm(b, qb, ctxps, cts):
                for j in range(2):
                    # free the accumulator bank after a single copy; the rest
                    # of the normalization runs from SBUF off the critical path
                    ctxf = ctxf_p.tile([HD + 1, 512], F32, name="ctxf")
                    nc.vector.tensor_copy(ctxf[:], ctxps[j][:])
                    ss = sums.tile([1, 512], F32, name="sums")
                    nc.vector.tensor_copy(ss[0:1, :], ctxf[HD:HD + 1, :])
                    rbb = rbp.tile([HD, 512], F32, name="rbb")
                    nc.gpsimd.partition_broadcast(rbb[:], ss[0:1, :])
                    rb = rbp.tile([HD, 512], F32, name="rb")
                    nc.vector.reciprocal_approx_fast(rb[:], rbb[:])
                    nc.vector.tensor_mul(
                        cts[64 * j:64 * (j + 1), qb * 512:(qb + 1) * 512],
                        ctxf[0:HD, :], rb[:])

            def emit_dense_qb(b, qb, cts):
                """Dense partial for the 512-token block qb (4 t-chunks)."""
                for t4 in range(4):
                    t = qb * 4 + t4
                    ob = outs.tile([128, H], F32, name="ostage")
                    for nb in range(2):
                        dp = ps_ms.tile([128, 512], F32, name="dense",
                                        tag="misc")
                        nc.tensor.matmul(
                            dp[:], cts[:, t * 128:(t + 1) * 128],
                            w2sb[:, nb * 512:(nb + 1) * 512],
                            start=True, stop=True)
                        nc.vector.tensor_copy(
                            ob[:, nb * 512:(nb + 1) * 512], dp[:])
                    row0 = b * S + t * 128
                    nc.sync.dma_start(out[row0:row0 + 128, :], ob[:])

            # ---- emission schedule ----
            # Serial qkv projection (DMA-paced; PE slack absorbs the v
            # transposes), then per-batch attention with dense interleaved
            # per query block so the epilogue never piles up at the end.
            for n in range(NN // 2):
                emit_qkv_nblock(n, fine=(n == 0))
            vbigs = {0: emit_vprep(0)}
            cts = {0: ctxp.tile([128, S], F32R, name="ctx_0")}
            pend = (0, 0, emit_attention_kc(0, 0, vbigs[0]))
            for n in range(NN // 2, NN):
                emit_qkv_nblock(n)
            vbigs[1] = emit_vprep(1)
            cts[1] = ctxp.tile([128, S], F32R, name="ctx_1")
            for b, qb in [(0, 1), (0, 2), (0, 3),
                          (1, 0), (1, 1), (1, 2), (1, 3)]:
                cur = (b, qb, emit_attention_kc(b, qb, vbigs[b]))
                pb, pq, pctx = pend
                emit_norm(pb, pq, pctx, cts[pb])
                emit_dense_qb(pb, pq, cts[pb])
                pend = cur
            pb, pq, pctx = pend
            emit_norm(pb, pq, pctx, cts[pb])
            emit_dense_qb(pb, pq, cts[pb])
    nc.compile()
    return nc


def _prepare_inputs(hidden_states, qkv_w, qkv_b, dense_w):
    """Build per-core input maps (all host-side slicing/transposition)."""
    x = np.ascontiguousarray(hidden_states, dtype=np.float32).reshape(BS, H)
    xt = _round_fp32r(np.ascontiguousarray(x.T))
    eye2 = np.concatenate([np.eye(HD, dtype=np.float32)] * 2, axis=0)
    ones2 = np.ones((128, HD), dtype=np.float32)
    in_maps = []
    for c in range(NCORES):
        base = c * ROWS_PER_CORE
        # per-head row groups within this core's 384 rows: h0 {q,k,v}, h1 {q,k,v}
        rows = {}
        for m in range(3):  # 0=q 1=k 2=v
            rows[m] = np.r_[base + m * HD:base + (m + 1) * HD,
                            base + 192 + m * HD:base + 192 + (m + 1) * HD]
        perm = np.concatenate([rows[0], rows[1], rows[2]])
        w1t = _round_fp32r(np.ascontiguousarray(qkv_w[perm, :].T))   # [H, 384]
        b1 = np.ascontiguousarray(
            np.stack([qkv_b[rows[m]] for m in range(3)], axis=1),
            dtype=np.float32)                                        # [128, 3]
        w2t0 = _round_fp32r(np.ascontiguousarray(
            dense_w[:, c * DPC:c * DPC + HD].T))                     # [64, 1024]
        w2t1 = _round_fp32r(np.ascontiguousarray(
            dense_w[:, c * DPC + HD:(c + 1) * DPC].T))
        in_maps.append({
            "xt": xt, "w1t": w1t, "b1": b1,
            "w2t0": w2t0, "w2t1": w2t1,
            "eye2": eye2, "ones2": ones2,
        })
    return in_maps


def _reference_numpy(hidden_states, attention_mask, qkv_w, qkv_b, dense_w, dense_b):
    """Exact fallback for non-all-ones masks (never hit with spec inputs)."""
    x = np.asarray(hidden_states, dtype=np.float64)
    mask = np.asarray(attention_mask, dtype=np.float64)
    mixed = x @ np.asarray(qkv_w, np.float64).T + np.asarray(qkv_b, np.float64)
    mixed = mixed.reshape(B, S, NH, 3 * HD).transpose(0, 2, 1, 3)
    q, k, v = np.split(mixed, 3, axis=-1)
    scores = np.einsum("bhqd,bhkd->bhqk", q, k) / np.sqrt(HD)
    scores = scores * mask - 10000.0 * (1.0 - mask)
    scores -= scores.max(axis=-1, keepdims=True)
    probs = np.exp(scores)
    probs /= probs.sum(axis=-1, keepdims=True)
    cx = np.einsum("bhqk,bhkd->bhqd", probs, v)
    cx = cx.transpose(0, 2, 1, 3).reshape(B, S, H)
    o = cx @ np.asarray(dense_w, np.float64).T + np.asarray(dense_b, np.float64)
    return o.astype(np.float32)


def _run(inputs, trace=False):
    from concourse.bass_utils import run_bass_kernel_spmd
    if "nc" not in _CACHE:
        _CACHE["nc"] = _build_program()
    nc = _CACHE["nc"]
    in_maps = _prepare_inputs(inputs["hidden_states"], inputs["qkv_w"],
                              inputs["qkv_b"], inputs["dense_w"])
    res = run_bass_kernel_spmd(nc, in_maps, core_ids=list(range(NCORES)),
                               trace=trace)
    partials = np.stack([r["out"] for r in res.results], axis=0)
    full = partials.sum(axis=0, dtype=np.float64)
    full += np.asarray(inputs["dense_b"], dtype=np.float64)
    return full.astype(np.float32).reshape(B, S, H), res


def kernel(hidden_states, attention_mask, qkv_w, qkv_b, dense_w, dense_b):
    hidden_states = np.asarray(hidden_states)
    attention_mask = np.asarray(attention_mask)
    qkv_w = np.asarray(qkv_w)
    qkv_b = np.asarray(qkv_b)
    dense_w = np.asarray(dense_w)
    dense_b = np.asarray(dense_b)
    if not np.all(attention_mask == 1.0):
        return _reference_numpy(hidden_states, attention_mask, qkv_w, qkv_b,
                                dense_w, dense_b)
    out, _ = _run({
        "hidden_states": hidden_states, "qkv_w": qkv_w, "qkv_b": qkv_b,
        "dense_w": dense_w, "dense_b": dense_b,
    }, trace=bool(int(os.environ.get("KERNEL_TRACE", "0"))))
    return out


# revision 26
# speedup vs baseline: 1.0738x; 1.0049x over previous
"""Trainium2 Bass kernel for nn_Attention: 16-head attention layer, B=2, S=2048, H=1024.

Strategy (Megatron-style tensor parallel over heads, 8 cores x 2 heads):
  - Host transposes hidden_states once (XT [H, B*S]) and pre-rounds all matmul
    inputs to fp32r (TF32-like: 11-bit mantissa) so every matmul runs at the
    full 1-cycle/row PE rate with fp32 accumulation.
  - Each core computes its 2 heads' q/k/v via XT @ its W slice (transposed
    layout), attention with softmax folded as exp -> matmul-rowsum -> late
    normalization, then a partial dense projection over its 128 ctx columns.
  - Host sums the 8 partial dense outputs and adds dense_b.

All computed on device except the final 8-way partial reduction (done at
gather time on host, per the Megatron all-reduce-after-dense recipe).
"""
import os
import numpy as np

B, S, H, NH = 2, 2048, 1024, 16
HD = H // NH            # 64
BS = B * S              # 4096
NCORES = 8
ROWS_PER_CORE = 3 * HD * 2   # 384 qkv rows per core
DPC = 2 * HD                 # 128 ctx/dense columns per core

_CACHE = {}


def _round_fp32r(x):
    bits = np.ascontiguousarray(x, dtype=np.float32).view(np.uint32)
    lsb = (bits >> np.uint32(12)) & np.uint32(1)
    return ((bits + np.uint32(0x7FF) + lsb) & np.uint32(0xFFFFF000)).view(np.float32)


def _build_program():
    import concourse.mybir as mybir
    import concourse.tile as tile
    from concourse import bacc

    F32 = mybir.dt.float32
    F32R = mybir.dt.float32r
    Act = mybir.ActivationFunctionType

    nc = bacc.Bacc("TRN2", target_bir_lowering=False, debug=False,
                   num_devices=NCORES)
    xt = nc.dram_tensor("xt", [H, BS], F32R, kind="ExternalInput").ap()
    w1t = nc.dram_tensor("w1t", [H, ROWS_PER_CORE], F32R, kind="ExternalInput").ap()
    b1 = nc.dram_tensor("b1", [128, 3], F32, kind="ExternalInput").ap()
    w2t0 = nc.dram_tensor("w2t0", [HD, H], F32R, kind="ExternalInput").ap()
    w2t1 = nc.dram_tensor("w2t1", [HD, H], F32R, kind="ExternalInput").ap()
    eye2 = nc.dram_tensor("eye2", [128, HD], F32R, kind="ExternalInput").ap()
    ones2 = nc.dram_tensor("ones2", [128, HD], F32R, kind="ExternalInput").ap()
    out = nc.dram_tensor("out", [BS, H], F32, kind="ExternalOutput").ap()

    NK = H // 128          # 8 contraction chunks for qkv
    NN = BS // 512         # 8 token blocks of 512
    NQB = S // 512         # 4 query blocks per batch
    NKC = S // 128         # 16 key chunks per batch

    with tile.TileContext(nc) as tc, nc.allow_low_precision(reason="fp32r"):
        from contextlib import ExitStack
        with ExitStack() as ctx:
            consts = ctx.enter_context(tc.tile_pool(name="consts", bufs=1))
            mixed = ctx.enter_context(tc.tile_pool(name="mixed", bufs=1))
            ctxp = ctx.enter_context(tc.tile_pool(name="ctxp", bufs=1))
            xtp = ctx.enter_context(tc.tile_pool(name="xtp", bufs=5))
            vsb = ctx.enter_context(tc.tile_pool(name="vsb", bufs=2))
            expp = ctx.enter_context(tc.tile_pool(name="expp", bufs=4))
            sums = ctx.enter_context(tc.tile_pool(name="sums", bufs=2))
            ctxf_p = ctx.enter_context(tc.tile_pool(name="ctxf", bufs=2))
            rbp = ctx.enter_context(tc.tile_pool(name="rbp", bufs=2))
            outs = ctx.enter_context(tc.tile_pool(name="outs", bufs=4))
            ps_sc = ctx.enter_context(tc.tile_pool(name="ps_sc", bufs=2, space="PSUM"))
            ps_ac = ctx.enter_context(tc.tile_pool(name="ps_ac", bufs=2, space="PSUM"))
            ps_ms = ctx.enter_context(tc.tile_pool(name="ps_ms", bufs=2, space="PSUM"))

            # ---- constants ----
            w1big = consts.tile([128, NK, ROWS_PER_CORE], F32R, name="w1big")
            w1r = w1t.rearrange("(k p) r -> p k r", p=128)
            nc.sync.dma_start(w1big[:, 0:1, :], w1r[:, 0:1, :])
            nc.sync.dma_start(w1big[:, 1:NK // 2, :], w1r[:, 1:NK // 2, :])
            nc.sync.dma_start(w1big[:, NK // 2:NK, :], w1r[:, NK // 2:NK, :])
            b1sb = consts.tile([128, 3], F32, name="b1")
            nc.sync.dma_start(b1sb[:], b1)
            warm = consts.tile([1, 1], F32, name="warm")
            nc.scalar.activation(warm[0:1, 0:1], b1sb[0:1, 0:1], Act.Exp)
            eye2sb = consts.tile([128, HD], F32R, name="eye2")
            nc.sync.dma_start(eye2sb[:], eye2)
            ones2sb = consts.tile([128, HD], F32R, name="ones2")
            nc.sync.dma_start(ones2sb[:], ones2)
            w2sb = consts.tile([128, H], F32R, name="w2pack")
            nc.sync.dma_start(w2sb[0:HD, :], w2t0)
            nc.sync.dma_start(w2sb[HD:128, :], w2t1)

            # ---- phase A building blocks ----
            qt = mixed.tile([128, BS], F32R, name="qt")
            kt = mixed.tile([128, BS], F32R, name="kt")
            vt = mixed.tile([128, BS], F32R, name="vt")
            mix_dst = [qt, kt, vt]
            KG = 4  # k-chunks per xt DMA

            def emit_qkv_nblock(n, fine=False):
                """mixedT[:, n*512:(n+1)*512] = W1 @ XT block (+bias).
                m-outer / k-inner: one PSUM slot at a time, PE K-contiguous.
                fine=True splits the loads per k-chunk so the first matmul
                starts as soon as 256KB has landed (kernel warmup)."""
                xts = []
                for kg in range(NK // KG):
                    xt_t = xtp.tile([128, KG, 512], F32R, name="xt")
                    if fine:
                        for c in range(KG):
                            k = kg * KG + c
                            nc.sync.dma_start(
                                xt_t[:, c, :],
                                xt[k * 128:(k + 1) * 128,
                                   n * 512:(n + 1) * 512])
                    else:
                        nc.sync.dma_start(
                            xt_t[:],
                            xt[kg * KG * 128:(kg + 1) * KG * 128,
                               n * 512:(n + 1) * 512].rearrange(
                                   "(c p) f -> p c f", p=128))
                    xts.append(xt_t)
                for m in range(3):
                    ps = ps_ac.tile([128, 512], F32, name=f"qkv{m}", tag="acc")
                    for k in range(NK):
                        nc.tensor.matmul(
                            ps[:],
                            w1big[:, k, m * 128:(m + 1) * 128],
                            xts[k // KG][:, k % KG, :],
                            start=(k == 0), stop=(k == NK - 1))
                    nc.scalar.activation(
                        mix_dst[m][:, n * 512:(n + 1) * 512], ps[:],
                        Act.Identity, bias=b1sb[:, m:m + 1])

            def emit_vprep(b):
                vbig = {}
                for j in range(2):
                    vb = vsb.tile([128, NKC * (HD + 1)], F32R, name=f"vbig{j}")
                    ones_view = vb[:].rearrange(
                        "p (c w) -> p c w", w=HD + 1)[:, :, HD:HD + 1]
                    nc.vector.tensor_copy(ones_view, ones2sb[:, 0:NKC])
                    for kc in range(NKC):
                        pt = ps_ms.tile([128, HD], F32R, name="vtr", tag="misc")
                        nc.tensor.transpose(
                            pt[:],
                            vt[64 * j:64 * j + 64,
                               b * S + kc * 128:b * S + (kc + 1) * 128],
                            eye2sb[64 * j:64 * j + 64, :])
                        nc.vector.tensor_copy(
                            vb[:, kc * (HD + 1):kc * (HD + 1) + HD], pt[:])
                    vbig[j] = vb
                return vbig

            def emit_attention_kc(b, qb, vbig):
                ctxps = {j: ps_ac.tile([HD + 1, 512], F32, name=f"ctxps{j}",
                                       tag="acc")
                         for j in range(2)}
                for kc in range(NKC):
                    sp2 = ps_sc.tile([128, 1024], F32, name="scores")
                    for j in range(2):
                        nc.tensor.matmul(
                            sp2[:, j * 512:(j + 1) * 512],
                            kt[64 * j:64 * j + 64,
                               b * S + kc * 128:b * S + (kc + 1) * 128],
                            qt[64 * j:64 * j + 64,
                               b * S + qb * 512:b * S + (qb + 1) * 512],
                            start=True, stop=True)
                    et2 = expp.tile([128, 1024], F32R, name="exp")
                    nc.scalar.activation(et2[:], sp2[:], Act.Exp, scale=0.125)
                    for j in range(2):
                        nc.tensor.matmul(
                            ctxps[j][:],
                            vbig[j][:, kc * (HD + 1):(kc + 1) * (HD + 1)],
                            et2[:, j * 512:(j + 1) * 512],
                            start=(kc == 0), stop=(kc == NKC - 1))
                return ctxps

            def emit_norm(b, qb, ctxps, cts):
                for j in range(2):
                    # free the accumulator bank after a single copy; the rest
                    # of the normalization runs from SBUF off the critical path
                    ctxf = ctxf_p.tile([HD + 1, 512], F32, name="ctxf")
                    nc.vector.tensor_copy(ctxf[:], ctxps[j][:])
                    ss = sums.tile([1, 512], F32, name="sums")
                    nc.vector.tensor_copy(ss[0:1, :], ctxf[HD:HD + 1, :])
                    rbb = rbp.tile([HD, 512], F32, name="rbb")
                    nc.gpsimd.partition_broadcast(rbb[:], ss[0:1, :])
                    rb = rbp.tile([HD, 512], F32, name="rb")
                    nc.vector.reciprocal_approx_fast(rb[:], rbb[:])
                    nc.vector.tensor_mul(
                        cts[64 * j:64 * (j + 1), qb * 512:(qb + 1) * 512],
                        ctxf[0:HD, :], rb[:])

            def emit_dense_qb(b, qb, cts):
                """Dense partial for the 512-token block qb (4 t-chunks)."""
                for t4 in range(4):
                    t = qb * 4 + t4
                    ob = outs.tile([128, H], F32, name="ostage")
                    for nb in range(2):
                        dp = ps_ms.tile([128, 512], F32, name="dense",
                                        tag="misc")
                        nc.tensor.matmul(
                            dp[:], cts[:, t * 128:(t + 1) * 128],
                            w2sb[:, nb * 512:(nb + 1) * 512],
                            start=True, stop=True)
                        nc.vector.tensor_copy(
                            ob[:, nb * 512:(nb + 1) * 512], dp[:])
                    row0 = b * S + t * 128
                    nc.sync.dma_start(out[row0:row0 + 128, :], ob[:])

            # ---- emission schedule ----
            # Serial qkv projection (DMA-paced; PE slack absorbs the v
            # transposes), then per-batch attention with dense interleaved
            # per query block so the epilogue never piles up at the end.
            for n in range(NN // 2):
                emit_qkv_nblock(n, fine=(n == 0))
            vbigs = {0: emit_vprep(0)}
            cts = {0: ctxp.tile([128, S], F32R, name="ctx_0")}
            pend = (0, 0, emit_attention_kc(0, 0, vbigs[0]))
            for n in range(NN // 2, NN):
                emit_qkv_nblock(n)
            cts[1] = ctxp.tile([128, S], F32R, name="ctx_1")
            for b, qb in [(0, 1), (0, 2), (0, 3),
                          (1, 0), (1, 1), (1, 2), (1, 3)]:
                if (b, qb) == (0, 2):
                    # batch-1 v transposes ride the attention window's spare
                    # PE/misc capacity instead of extending phase A
                    vbigs[1] = emit_vprep(1)
                cur = (b, qb, emit_attention_kc(b, qb, vbigs[b]))
                pb, pq, pctx = pend
                emit_norm(pb, pq, pctx, cts[pb])
                emit_dense_qb(pb, pq, cts[pb])
                pend = cur
            pb, pq, pctx = pend
            emit_norm(pb, pq, pctx, cts[pb])
            emit_dense_qb(pb, pq, cts[pb])
    nc.compile()
    return nc


def _prepare_inputs(hidden_states, qkv_w, qkv_b, dense_w):
    """Build per-core input maps (all host-side slicing/transposition)."""
    x = np.ascontiguousarray(hidden_states, dtype=np.float32).reshape(BS, H)
    xt = _round_fp32r(np.ascontiguousarray(x.T))
    eye2 = np.concatenate([np.eye(HD, dtype=np.float32)] * 2, axis=0)
    ones2 = np.ones((128, HD), dtype=np.float32)
    in_maps = []
    for c in range(NCORES):
        base = c * ROWS_PER_CORE
        # per-head row groups within this core's 384 rows: h0 {q,k,v}, h1 {q,k,v}
        rows = {}
        for m in range(3):  # 0=q 1=k 2=v
            rows[m] = np.r_[base + m * HD:base + (m + 1) * HD,
                            base + 192 + m * HD:base + 192 + (m + 1) * HD]
        perm = np.concatenate([rows[0], rows[1], rows[2]])
        w1t = _round_fp32r(np.ascontiguousarray(qkv_w[perm, :].T))   # [H, 384]
        b1 = np.ascontiguousarray(
            np.stack([qkv_b[rows[m]] for m in range(3)], axis=1),
            dtype=np.float32)                                        # [128, 3]
        w2t0 = _round_fp32r(np.ascontiguousarray(
            dense_w[:, c * DPC:c * DPC + HD].T))                     # [64, 1024]
        w2t1 = _round_fp32r(np.ascontiguousarray(
            dense_w[:, c * DPC + HD:(c + 1) * DPC].T))
        in_maps.append({
            "xt": xt, "w1t": w1t, "b1": b1,
            "w2t0": w2t0, "w2t1": w2t1,
            "eye2": eye2, "ones2": ones2,
        })
    return in_maps


def _reference_numpy(hidden_states, attention_mask, qkv_w, qkv_b, dense_w, dense_b):
    """Exact fallback for non-all-ones masks (never hit with spec inputs)."""
    x = np.asarray(hidden_states, dtype=np.float64)
    mask = np.asarray(attention_mask, dtype=np.float64)
    mixed = x @ np.asarray(qkv_w, np.float64).T + np.asarray(qkv_b, np.float64)
    mixed = mixed.reshape(B, S, NH, 3 * HD).transpose(0, 2, 1, 3)
    q, k, v = np.split(mixed, 3, axis=-1)
    scores = np.einsum("bhqd,bhkd->bhqk", q, k) / np.sqrt(HD)
    scores = scores * mask - 10000.0 * (1.0 - mask)
    scores -= scores.max(axis=-1, keepdims=True)
    probs = np.exp(scores)
    probs /= probs.sum(axis=-1, keepdims=True)
    cx = np.einsum("bhqk,bhkd->bhqd", probs, v)
    cx = cx.transpose(0, 2, 1, 3).reshape(B, S, H)
    o = cx @ np.asarray(dense_w, np.float64).T + np.asarray(dense_b, np.float64)
    return o.astype(np.float32)


def _run(inputs, trace=False):
    from concourse.bass_utils import run_bass_kernel_spmd
    if "nc" not in _CACHE:
        _CACHE["nc"] = _build_program()
    nc = _CACHE["nc"]
    in_maps = _prepare_inputs(inputs["hidden_states"], inputs["qkv_w"],
                              inputs["qkv_b"], inputs["dense_w"])
    res = run_bass_kernel_spmd(nc, in_maps, core_ids=list(range(NCORES)),
                               trace=trace)
    partials = np.stack([r["out"] for r in res.results], axis=0)
    full = partials.sum(axis=0, dtype=np.float64)
    full += np.asarray(inputs["dense_b"], dtype=np.float64)
    return full.astype(np.float32).reshape(B, S, H), res


def kernel(hidden_states, attention_mask, qkv_w, qkv_b, dense_w, dense_b):
    hidden_states = np.asarray(hidden_states)
    attention_mask = np.asarray(attention_mask)
    qkv_w = np.asarray(qkv_w)
    qkv_b = np.asarray(qkv_b)
    dense_w = np.asarray(dense_w)
    dense_b = np.asarray(dense_b)
    if not np.all(attention_mask == 1.0):
        return _reference_numpy(hidden_states, attention_mask, qkv_w, qkv_b,
                                dense_w, dense_b)
    out, _ = _run({
        "hidden_states": hidden_states, "qkv_w": qkv_w, "qkv_b": qkv_b,
        "dense_w": dense_w, "dense_b": dense_b,
    }, trace=bool(int(os.environ.get("KERNEL_TRACE", "0"))))
    return out


# revision 29
# speedup vs baseline: 1.0846x; 1.0100x over previous
"""Trainium2 Bass kernel for nn_Attention: 16-head attention layer, B=2, S=2048, H=1024.

Strategy (Megatron-style tensor parallel over heads, 8 cores x 2 heads):
  - Host transposes hidden_states once (XT [H, B*S]) and pre-rounds all matmul
    inputs to fp32r (TF32-like: 11-bit mantissa) so every matmul runs at the
    full 1-cycle/row PE rate with fp32 accumulation.
  - Each core computes its 2 heads' q/k/v via XT @ its W slice (transposed
    layout), attention with softmax folded as exp -> matmul-rowsum -> late
    normalization, then a partial dense projection over its 128 ctx columns.
  - Host sums the 8 partial dense outputs and adds dense_b.

All computed on device except the final 8-way partial reduction (done at
gather time on host, per the Megatron all-reduce-after-dense recipe).
"""
import os
import numpy as np

B, S, H, NH = 2, 2048, 1024, 16
HD = H // NH            # 64
BS = B * S              # 4096
NCORES = 8
ROWS_PER_CORE = 3 * HD * 2   # 384 qkv rows per core
DPC = 2 * HD                 # 128 ctx/dense columns per core

_CACHE = {}


def _round_fp32r(x):
    bits = np.ascontiguousarray(x, dtype=np.float32).view(np.uint32)
    lsb = (bits >> np.uint32(12)) & np.uint32(1)
    return ((bits + np.uint32(0x7FF) + lsb) & np.uint32(0xFFFFF000)).view(np.float32)


def _build_program():
    import concourse.mybir as mybir
    import concourse.tile as tile
    from concourse import bacc

    F32 = mybir.dt.float32
    F32R = mybir.dt.float32r
    Act = mybir.ActivationFunctionType

    nc = bacc.Bacc("TRN2", target_bir_lowering=False, debug=False,
                   num_devices=NCORES)
    xt = nc.dram_tensor("xt", [H, BS], F32R, kind="ExternalInput").ap()
    w1t = nc.dram_tensor("w1t", [H, ROWS_PER_CORE], F32R, kind="ExternalInput").ap()
    b1 = nc.dram_tensor("b1", [128, 3], F32, kind="ExternalInput").ap()
    w2t0 = nc.dram_tensor("w2t0", [HD, H], F32R, kind="ExternalInput").ap()
    w2t1 = nc.dram_tensor("w2t1", [HD, H], F32R, kind="ExternalInput").ap()
    eye2 = nc.dram_tensor("eye2", [128, HD], F32R, kind="ExternalInput").ap()
    ones2 = nc.dram_tensor("ones2", [128, HD], F32R, kind="ExternalInput").ap()
    out = nc.dram_tensor("out", [BS, H], F32, kind="ExternalOutput").ap()

    NK = H // 128          # 8 contraction chunks for qkv
    NN = BS // 512         # 8 token blocks of 512
    NQB = S // 512         # 4 query blocks per batch
    NKC = S // 128         # 16 key chunks per batch

    with tile.TileContext(nc) as tc, nc.allow_low_precision(reason="fp32r"):
        from contextlib import ExitStack
        with ExitStack() as ctx:
            consts = ctx.enter_context(tc.tile_pool(name="consts", bufs=1))
            mixed = ctx.enter_context(tc.tile_pool(name="mixed", bufs=1))
            ctxp = ctx.enter_context(tc.tile_pool(name="ctxp", bufs=1))
            xtp = ctx.enter_context(tc.tile_pool(name="xtp", bufs=5))
            vsb = ctx.enter_context(tc.tile_pool(name="vsb", bufs=2))
            expp = ctx.enter_context(tc.tile_pool(name="expp", bufs=9))
            sums = ctx.enter_context(tc.tile_pool(name="sums", bufs=2))
            ctxf_p = ctx.enter_context(tc.tile_pool(name="ctxf", bufs=2))
            rbp = ctx.enter_context(tc.tile_pool(name="rbp", bufs=2))
            outs = ctx.enter_context(tc.tile_pool(name="outs", bufs=4))
            ps_sc = ctx.enter_context(tc.tile_pool(name="ps_sc", bufs=2, space="PSUM"))
            ps_ac = ctx.enter_context(tc.tile_pool(name="ps_ac", bufs=2, space="PSUM"))
            ps_ms = ctx.enter_context(tc.tile_pool(name="ps_ms", bufs=2, space="PSUM"))

            # ---- constants ----
            w1big = consts.tile([128, NK, ROWS_PER_CORE], F32R, name="w1big")
            w1r = w1t.rearrange("(k p) r -> p k r", p=128)
            nc.sync.dma_start(w1big[:, 0:1, :], w1r[:, 0:1, :])
            nc.sync.dma_start(w1big[:, 1:NK // 2, :], w1r[:, 1:NK // 2, :])
            nc.sync.dma_start(w1big[:, NK // 2:NK, :], w1r[:, NK // 2:NK, :])
            b1sb = consts.tile([128, 3], F32, name="b1")
            nc.sync.dma_start(b1sb[:], b1)
            warm = consts.tile([1, 1], F32, name="warm")
            nc.scalar.activation(warm[0:1, 0:1], b1sb[0:1, 0:1], Act.Exp)
            eye2sb = consts.tile([128, HD], F32R, name="eye2")
            nc.sync.dma_start(eye2sb[:], eye2)
            ones2sb = consts.tile([128, HD], F32R, name="ones2")
            nc.sync.dma_start(ones2sb[:], ones2)
            w2sb = consts.tile([128, H], F32R, name="w2pack")
            nc.sync.dma_start(w2sb[0:HD, :], w2t0)
            nc.sync.dma_start(w2sb[HD:128, :], w2t1)

            # ---- phase A building blocks ----
            qt = mixed.tile([128, BS], F32R, name="qt")
            kt = mixed.tile([128, BS], F32R, name="kt")
            vt = mixed.tile([128, BS], F32R, name="vt")
            mix_dst = [qt, kt, vt]
            KG = 4  # k-chunks per xt DMA

            def emit_qkv_nblock(n, fine=False):
                """mixedT[:, n*512:(n+1)*512] = W1 @ XT block (+bias).
                m-outer / k-inner: one PSUM slot at a time, PE K-contiguous.
                fine=True splits the loads per k-chunk so the first matmul
                starts as soon as 256KB has landed (kernel warmup)."""
                xts = []
                for kg in range(NK // KG):
                    xt_t = xtp.tile([128, KG, 512], F32R, name="xt")
                    if fine:
                        for c in range(KG):
                            k = kg * KG + c
                            nc.sync.dma_start(
                                xt_t[:, c, :],
                                xt[k * 128:(k + 1) * 128,
                                   n * 512:(n + 1) * 512])
                    else:
                        nc.sync.dma_start(
                            xt_t[:],
                            xt[kg * KG * 128:(kg + 1) * KG * 128,
                               n * 512:(n + 1) * 512].rearrange(
                                   "(c p) f -> p c f", p=128))
                    xts.append(xt_t)
                for m in range(3):
                    ps = ps_ac.tile([128, 512], F32, name=f"qkv{m}", tag="acc")
                    for k in range(NK):
                        nc.tensor.matmul(
                            ps[:],
                            w1big[:, k, m * 128:(m + 1) * 128],
                            xts[k // KG][:, k % KG, :],
                            start=(k == 0), stop=(k == NK - 1))
                    nc.scalar.activation(
                        mix_dst[m][:, n * 512:(n + 1) * 512], ps[:],
                        Act.Identity, bias=b1sb[:, m:m + 1])

            def emit_vprep(b):
                vbig = {}
                for j in range(2):
                    vb = vsb.tile([128, NKC * (HD + 1)], F32R, name=f"vbig{j}")
                    ones_view = vb[:].rearrange(
                        "p (c w) -> p c w", w=HD + 1)[:, :, HD:HD + 1]
                    nc.vector.tensor_copy(ones_view, ones2sb[:, 0:NKC])
                    for kc in range(NKC):
                        pt = ps_ms.tile([128, HD], F32R, name="vtr", tag="misc")
                        nc.tensor.transpose(
                            pt[:],
                            vt[64 * j:64 * j + 64,
                               b * S + kc * 128:b * S + (kc + 1) * 128],
                            eye2sb[64 * j:64 * j + 64, :])
                        nc.vector.tensor_copy(
                            vb[:, kc * (HD + 1):kc * (HD + 1) + HD], pt[:])
                    vbig[j] = vb
                return vbig

            def emit_attention_kc(b, qb, vbig):
                ctxps = {j: ps_ac.tile([HD + 1, 512], F32, name=f"ctxps{j}",
                                       tag="acc")
                         for j in range(2)}
                for kc in range(NKC):
                    sp2 = ps_sc.tile([128, 1024], F32, name="scores")
                    for j in range(2):
                        nc.tensor.matmul(
                            sp2[:, j * 512:(j + 1) * 512],
                            kt[64 * j:64 * j + 64,
                               b * S + kc * 128:b * S + (kc + 1) * 128],
                            qt[64 * j:64 * j + 64,
                               b * S + qb * 512:b * S + (qb + 1) * 512],
                            start=True, stop=True)
                    et2 = expp.tile([128, 1024], F32R, name="exp")
                    nc.scalar.activation(et2[:], sp2[:], Act.Exp, scale=0.125)
                    for j in range(2):
                        nc.tensor.matmul(
                            ctxps[j][:],
                            vbig[j][:, kc * (HD + 1):(kc + 1) * (HD + 1)],
                            et2[:, j * 512:(j + 1) * 512],
                            start=(kc == 0), stop=(kc == NKC - 1))
                return ctxps

            def emit_norm(b, qb, ctxps, cts):
                for j in range(2):
                    # free the accumulator bank after a single copy; the rest
                    # of the normalization runs from SBUF off the critical path
                    ctxf = ctxf_p.tile([HD + 1, 512], F32, name="ctxf")
                    nc.vector.tensor_copy(ctxf[:], ctxps[j][:])
                    ss = sums.tile([1, 512], F32, name="sums")
                    nc.vector.tensor_copy(ss[0:1, :], ctxf[HD:HD + 1, :])
                    rbb = rbp.tile([HD, 512], F32, name="rbb")
                    nc.gpsimd.partition_broadcast(rbb[:], ss[0:1, :])
                    rb = rbp.tile([HD, 512], F32, name="rb")
                    nc.vector.reciprocal_approx_fast(rb[:], rbb[:])
                    nc.vector.tensor_mul(
                        cts[64 * j:64 * (j + 1), qb * 512:(qb + 1) * 512],
                        ctxf[0:HD, :], rb[:])

            def emit_dense_qb(b, qb, cts):
                """Dense partial for the 512-token block qb (4 t-chunks)."""
                for t4 in range(4):
                    t = qb * 4 + t4
                    ob = outs.tile([128, H], F32, name="ostage")
                    for nb in range(2):
                        dp = ps_ms.tile([128, 512], F32, name="dense",
                                        tag="misc")
                        nc.tensor.matmul(
                            dp[:], cts[:, t * 128:(t + 1) * 128],
                            w2sb[:, nb * 512:(nb + 1) * 512],
                            start=True, stop=True)
                        nc.vector.tensor_copy(
                            ob[:, nb * 512:(nb + 1) * 512], dp[:])
                    row0 = b * S + t * 128
                    nc.sync.dma_start(out[row0:row0 + 128, :], ob[:])

            # ---- emission schedule ----
            # Serial qkv projection (DMA-paced; PE slack absorbs the v
            # transposes), then per-batch attention with dense interleaved
            # per query block so the epilogue never piles up at the end.
            for n in range(NN // 2):
                emit_qkv_nblock(n, fine=(n == 0))
            vbigs = {0: emit_vprep(0)}
            cts = {0: ctxp.tile([128, S], F32R, name="ctx_0")}
            pend = (0, 0, emit_attention_kc(0, 0, vbigs[0]))
            for n in range(NN // 2, NN):
                emit_qkv_nblock(n)
            cts[1] = ctxp.tile([128, S], F32R, name="ctx_1")
            for b, qb in [(0, 1), (0, 2), (0, 3),
                          (1, 0), (1, 1), (1, 2), (1, 3)]:
                if (b, qb) == (0, 2):
                    # batch-1 v transposes ride the attention window's spare
                    # PE/misc capacity instead of extending phase A
                    vbigs[1] = emit_vprep(1)
                cur = (b, qb, emit_attention_kc(b, qb, vbigs[b]))
                pb, pq, pctx = pend
                emit_norm(pb, pq, pctx, cts[pb])
                emit_dense_qb(pb, pq, cts[pb])
                pend = cur
            pb, pq, pctx = pend
            emit_norm(pb, pq, pctx, cts[pb])
            emit_dense_qb(pb, pq, cts[pb])
    nc.compile()
    return nc


def _prepare_inputs(hidden_states, qkv_w, qkv_b, dense_w):
    """Build per-core input maps (all host-side slicing/transposition)."""
    x = np.ascontiguousarray(hidden_states, dtype=np.float32).reshape(BS, H)
    xt = _round_fp32r(np.ascontiguousarray(x.T))
    eye2 = np.concatenate([np.eye(HD, dtype=np.float32)] * 2, axis=0)
    ones2 = np.ones((128, HD), dtype=np.float32)
    in_maps = []
    for c in range(NCORES):
        base = c * ROWS_PER_CORE
        # per-head row groups within this core's 384 rows: h0 {q,k,v}, h1 {q,k,v}
        rows = {}
        for m in range(3):  # 0=q 1=k 2=v
            rows[m] = np.r_[base + m * HD:base + (m + 1) * HD,
                            base + 192 + m * HD:base + 192 + (m + 1) * HD]
        perm = np.concatenate([rows[0], rows[1], rows[2]])
        w1t = _round_fp32r(np.ascontiguousarray(qkv_w[perm, :].T))   # [H, 384]
        b1 = np.ascontiguousarray(
            np.stack([qkv_b[rows[m]] for m in range(3)], axis=1),
            dtype=np.float32)                                        # [128, 3]
        w2t0 = _round_fp32r(np.ascontiguousarray(
            dense_w[:, c * DPC:c * DPC + HD].T))                     # [64, 1024]
        w2t1 = _round_fp32r(np.ascontiguousarray(
            dense_w[:, c * DPC + HD:(c + 1) * DPC].T))
        in_maps.append({
            "xt": xt, "w1t": w1t, "b1": b1,
            "w2t0": w2t0, "w2t1": w2t1,
            "eye2": eye2, "ones2": ones2,
        })
    return in_maps


def _reference_numpy(hidden_states, attention_mask, qkv_w, qkv_b, dense_w, dense_b):
    """Exact fallback for non-all-ones masks (never hit with spec inputs)."""
    x = np.asarray(hidden_states, dtype=np.float64)
    mask = np.asarray(attention_mask, dtype=np.float64)
    mixed = x @ np.asarray(qkv_w, np.float64).T + np.asarray(qkv_b, np.float64)
    mixed = mixed.reshape(B, S, NH, 3 * HD).transpose(0, 2, 1, 3)
    q, k, v = np.split(mixed, 3, axis=-1)
    scores = np.einsum("bhqd,bhkd->bhqk", q, k) / np.sqrt(HD)
    scores = scores * mask - 10000.0 * (1.0 - mask)
    scores -= scores.max(axis=-1, keepdims=True)
    probs = np.exp(scores)
    probs /= probs.sum(axis=-1, keepdims=True)
    cx = np.einsum("bhqk,bhkd->bhqd", probs, v)
    cx = cx.transpose(0, 2, 1, 3).reshape(B, S, H)
    o = cx @ np.asarray(dense_w, np.float64).T + np.asarray(dense_b, np.float64)
    return o.astype(np.float32)


def _run(inputs, trace=False):
    from concourse.bass_utils import run_bass_kernel_spmd
    if "nc" not in _CACHE:
        _CACHE["nc"] = _build_program()
    nc = _CACHE["nc"]
    in_maps = _prepare_inputs(inputs["hidden_states"], inputs["qkv_w"],
                              inputs["qkv_b"], inputs["dense_w"])
    res = run_bass_kernel_spmd(nc, in_maps, core_ids=list(range(NCORES)),
                               trace=trace)
    partials = np.stack([r["out"] for r in res.results], axis=0)
    full = partials.sum(axis=0, dtype=np.float64)
    full += np.asarray(inputs["dense_b"], dtype=np.float64)
    return full.astype(np.float32).reshape(B, S, H), res


def kernel(hidden_states, attention_mask, qkv_w, qkv_b, dense_w, dense_b):
    hidden_states = np.asarray(hidden_states)
    attention_mask = np.asarray(attention_mask)
    qkv_w = np.asarray(qkv_w)
    qkv_b = np.asarray(qkv_b)
    dense_w = np.asarray(dense_w)
    dense_b = np.asarray(dense_b)
    if not np.all(attention_mask == 1.0):
        return _reference_numpy(hidden_states, attention_mask, qkv_w, qkv_b,
                                dense_w, dense_b)
    out, _ = _run({
        "hidden_states": hidden_states, "qkv_w": qkv_w, "qkv_b": qkv_b,
        "dense_w": dense_w, "dense_b": dense_b,
    }, trace=bool(int(os.environ.get("KERNEL_TRACE", "0"))))
    return out
